# revision 7
# baseline (speedup 1.0000x reference)
"""MiniMaxText01 linear attention layer on 8 Trainium2 NeuronCores.

Tensor-parallel over heads (4 heads per core). Per core:
  - fused QKV+gate projection computed in transposed layout
    (features on partitions, sequence on free dim) with bf16 matmuls
  - lightning (chunked linear) attention with per-head decay, two heads
    packed per 128-partition group (PE row/col tiling)
  - RMSNorm variance via ones-matmul partition reduction + one 32KB
    AllReduce across the 8 cores; rsqrt folded into the gated hidden
  - out-proj row-parallel: each core emits a full-width partial output
    (transposed); host sums the 8 partials and transposes back.

Everything is hardcoded for the fixed problem shapes below.
"""

import math
import warnings

warnings.filterwarnings("ignore")

import numpy as np
import ml_dtypes

import concourse.bacc as bacc
import concourse.mybir as mybir
import concourse.tile as tile
from concourse.bass_utils import run_bass_kernel_spmd
from concourse.masks import make_identity

F32 = mybir.dt.float32
BF16 = mybir.dt.bfloat16
BF = ml_dtypes.bfloat16
AF = mybir.ActivationFunctionType

N = 8192          # sequence length
HID = 2048        # hidden size
H = 32            # total heads
D = 64            # head dim
BLOCK = 256       # attention chunk size
NCORES = 8
HL = H // NCORES  # 4 local heads per core
CHUNK = 512       # seq columns processed per projection chunk
NCHUNK = N // CHUNK
BPC = CHUNK // BLOCK  # blocks per chunk
EPS = 1e-5
NUM_LAYERS, LAYER_IDX = 80, 0

LAST_EXEC_NS = None
LAST_RESULTS = None


def _build_slopes(n):
    def p2(m):
        start = 2 ** (-(2 ** (-(math.log2(m) - 3))))
        return [start * start**i for i in range(m)]

    if math.log2(n).is_integer():
        s = p2(n)
    else:
        cp = 2 ** math.floor(math.log2(n))
        s = p2(cp) + _build_slopes(2 * cp).tolist()[0::2][: n - cp]
    return np.array(s, dtype=np.float32)


SLOPE = _build_slopes(H) * (1.0 - LAYER_IDX / (NUM_LAYERS - 1) + 1e-5)  # [H]

_NC_CACHE = None


def _build_module():
    nc = bacc.Bacc("TRN2", target_bir_lowering=False, num_devices=NCORES)

    hsT_d = nc.dram_tensor("hsT", [HID, N], BF16, kind="ExternalInput")
    wc_d = nc.dram_tensor("wcomb", [HID, 2 * CHUNK], BF16, kind="ExternalInput")
    wo_d = nc.dram_tensor("wout", [2 * 128, HID], BF16, kind="ExternalInput")
    dd_d = nc.dram_tensor("dd", [128, 2 * HL, BLOCK], F32, kind="ExternalInput")
    qd_d = nc.dram_tensor("qd", [128, HL // 2, BLOCK], BF16, kind="ExternalInput")
    kd_d = nc.dram_tensor("kd", [128, 2 * HL], F32, kind="ExternalInput")
    bd_d = nc.dram_tensor("bd", [128, HL // 2], F32, kind="ExternalInput")
    kv0_d = nc.dram_tensor("kv0", [128, HL // 2, D], F32, kind="ExternalInput")
    outp_d = nc.dram_tensor("outp", [HID, N], F32, kind="ExternalOutput")

    with tile.TileContext(nc) as tc:
        with (
            tc.tile_pool(name="singles", bufs=1) as sg,
            tc.tile_pool(name="hstp", bufs=2) as hstp,
            tc.tile_pool(name="combp", bufs=2) as combp,
            tc.tile_pool(name="gatep", bufs=2) as gatep,
            tc.tile_pool(name="kvp", bufs=3) as kvp,
            tc.tile_pool(name="qpp", bufs=3) as qpp,
            tc.tile_pool(name="qkpp", bufs=3) as qkpp,
            tc.tile_pool(name="hsqp", bufs=2) as hsqp,
            tc.tile_pool(name="osbp", bufs=3) as osbp,
            tc.tile_pool(name="psA", bufs=2, space="PSUM") as psA,
            tc.tile_pool(name="psB", bufs=2, space="PSUM") as psB,
            tc.tile_pool(name="psC", bufs=3, space="PSUM") as psC,
            tc.tile_pool(name="psV", bufs=1, space="PSUM") as psV,
            tc.tile_pool(name="dram", bufs=1, space="DRAM") as dram,
        ):
            # ---- resident tensors -------------------------------------
            wc_sb = sg.tile([128, HID // 128, 2 * CHUNK], BF16)
            nc.sync.dma_start(
                out=wc_sb, in_=wc_d[:].rearrange("(kt p) m -> p kt m", p=128)
            )
            wo_sb = sg.tile([128, 2, HID], BF16)
            nc.sync.dma_start(
                out=wo_sb, in_=wo_d[:].rearrange("(kt p) m -> p kt m", p=128)
            )
            gh_sb = sg.tile([128, 2, N], BF16)
            dd_sb = sg.tile([128, 2 * HL, BLOCK], F32)
            nc.sync.dma_start(out=dd_sb, in_=dd_d[:])
            qd_sb = sg.tile([128, HL // 2, BLOCK], BF16)
            nc.sync.dma_start(out=qd_sb, in_=qd_d[:])
            kd_sb = sg.tile([128, 2 * HL], F32)
            nc.sync.dma_start(out=kd_sb, in_=kd_d[:])
            bd_sb = sg.tile([128, HL // 2], F32)
            nc.sync.dma_start(out=bd_sb, in_=bd_d[:])
            S32 = sg.tile([128, HL // 2, D], F32)
            nc.sync.dma_start(out=S32, in_=kv0_d[:])
            Sbf = sg.tile([128, HL // 2, D], BF16)
            for p in range(HL // 2):
                nc.vector.tensor_copy(Sbf[:, p, :], S32[:, p, :])
            ones_sb = sg.tile([128, 1], BF16)
            nc.vector.memset(ones_sb, 1.0)
            eps_sb = sg.tile([128, 1], F32)
            nc.vector.memset(eps_sb, EPS)
            ident = sg.tile([128, 128], BF16)
            make_identity(nc, ident)
            rs64 = sg.tile([128, N // 128], F32)
            tmp64 = sg.tile([128, N // 128], F32)
            r64 = sg.tile([128, N // 128], F32)

            cc_in = dram.tile([1, N], F32)
            cc_out = dram.tile([1, N], F32)

            hsT_r = hsT_d[:].rearrange("(kt p) s -> p kt s", p=128)

            # ---- main pass: projection + attention --------------------
            for c in range(NCHUNK):
                C0 = c * CHUNK
                hst = hstp.tile([128, HID // 128, CHUNK], BF16, name="hst")
                nc.sync.dma_start(out=hst, in_=hsT_r[:, :, C0 : C0 + CHUNK])

                comb = combp.tile([128, 6, CHUNK], BF16, name="comb")
                gate = gatep.tile([128, 2, CHUNK], F32, name="gate")

                for mt in range(8):
                    pj = psA.tile([128, CHUNK], F32, tag="pj", name="pj")
                    for kt in range(HID // 128):
                        nc.tensor.matmul(
                            pj,
                            lhsT=wc_sb[:, kt, mt * 128 : (mt + 1) * 128],
                            rhs=hst[:, kt, :],
                            start=(kt == 0),
                            stop=(kt == HID // 128 - 1),
                        )
                    if mt < 6:
                        nc.scalar.activation(comb[:, mt, :], pj, AF.Silu)
                    else:
                        nc.scalar.activation(gate[:, mt - 6, :], pj, AF.Sigmoid)

                ps_var = psV.tile([1, CHUNK], F32, tag="var", name="ps_var")

                for blk in range(BPC):
                    bc = blk * BLOCK
                    gc = C0 + bc
                    kn = {}
                    vn = {}
                    # k/v natural layout via PE transpose (both heads of a
                    # pair at once: [128 seq-half, 128 = h_even d | h_odd d])
                    for p in range(2):
                        knt = kvp.tile([128, 2, 128], BF16, tag="kn", name="knt")
                        vnt = kvp.tile([128, 2, 128], BF16, tag="vn", name="vnt")
                        kn[p] = knt
                        vn[p] = vnt
                        for half in range(2):
                            tpk = psB.tile([128, 256], BF16, tag="qk", name="tpk")
                            nc.tensor.transpose(
                                tpk[:, 0:128],
                                in_=comb[:, 2 + p, bc + half * 128 : bc + (half + 1) * 128],
                                identity=ident,
                            )
                            for hi in range(2):
                                h = 2 * p + hi
                                nc.vector.tensor_scalar_mul(
                                    knt[:, half, hi * 64 : hi * 64 + 64],
                                    tpk[:, hi * 64 : hi * 64 + 64],
                                    kd_sb[:, 2 * h + half : 2 * h + half + 1],
                                )
                            tpv = psB.tile([128, 256], BF16, tag="qk", name="tpv")
                            nc.tensor.transpose(
                                tpv[:, 0:128],
                                in_=comb[:, 4 + p, bc + half * 128 : bc + (half + 1) * 128],
                                identity=ident,
                            )
                            nc.scalar.copy(vnt[:, half, :], tpv[:, 0:128])

                    for p in range(2):
                        # decayed queries for the pair
                        qp = qpp.tile([128, BLOCK], BF16, tag="qp", name="qp")
                        for hi in range(2):
                            b = hi * 64
                            nc.vector.tensor_mul(
                                qp[b : b + 64, :],
                                comb[b : b + 64, p, bc : bc + BLOCK],
                                qd_sb[b : b + 64, p, :],
                            )
                        # scores (transposed) + decay mask
                        qkp = {}
                        for hi in range(2):
                            h = 2 * p + hi
                            b = hi * 64
                            qkph = qkpp.tile([128, 2, BLOCK], BF16, tag="qkp", name="qkph")
                            qkp[hi] = qkph
                            for half in range(2):
                                qk_ps = psB.tile([128, 256], F32, tag="qk", name="qk_ps")
                                nc.tensor.matmul(
                                    qk_ps,
                                    lhsT=comb[b : b + 64, 2 + p, bc + half * 128 : bc + (half + 1) * 128],
                                    rhs=comb[b : b + 64, p, bc : bc + BLOCK],
                                    start=True,
                                    stop=True,
                                    tile_position=(b, 0),
                                )
                                nc.vector.tensor_mul(
                                    qkph[:, half, :], qk_ps, dd_sb[:, 2 * h + half, :]
                                )
                        # attention output (transposed): inter + intra
                        po = psC.tile([128, BLOCK], F32, tag="po", name="po")
                        for hi in range(2):
                            b = hi * 64
                            nc.tensor.matmul(
                                po[b : b + 64, :],
                                lhsT=Sbf[b : b + 64, p, :],
                                rhs=qp[b : b + 64, :],
                                start=True,
                                stop=False,
                                tile_position=(b, b),
                            )
                            for half in range(2):
                                nc.tensor.matmul(
                                    po[b : b + 64, :],
                                    lhsT=vn[p][:, half, b : b + 64],
                                    rhs=qkp[hi][:, half, :],
                                    start=False,
                                    stop=(half == 1),
                                    tile_position=(0, b),
                                )
                        # sum of squares for RMSNorm variance
                        hsq = hsqp.tile([128, BLOCK], BF16, tag="hsq", name="hsq")
                        nc.scalar.square(hsq, po)
                        nc.tensor.matmul(
                            ps_var[0:1, bc : bc + BLOCK],
                            lhsT=ones_sb,
                            rhs=hsq,
                            start=(p == 0),
                            stop=(p == 1),
                        )
                        # gated hidden (pre-normalization)
                        nc.vector.tensor_mul(
                            gh_sb[:, p, gc : gc + BLOCK], po, gate[:, p, bc : bc + BLOCK]
                        )
                        # state update S = bd*S + k'^T v
                        psS = psC.tile([128, D], F32, tag="po", name="psS")
                        for hi in range(2):
                            b = hi * 64
                            for half in range(2):
                                nc.tensor.matmul(
                                    psS[b : b + 64, :],
                                    lhsT=kn[p][:, half, b : b + 64],
                                    rhs=vn[p][:, half, b : b + 64],
                                    start=(half == 0),
                                    stop=(half == 1),
                                    tile_position=(0, b),
                                )
                        nc.vector.tensor_scalar_mul(
                            S32[:, p, :], S32[:, p, :], bd_sb[:, p : p + 1]
                        )
                        nc.vector.tensor_add(S32[:, p, :], S32[:, p, :], psS)
                        nc.vector.tensor_copy(Sbf[:, p, :], S32[:, p, :])

                ssqc = hsqp.tile([1, CHUNK], F32, tag="ssqc", name="ssqc")
                nc.scalar.copy(ssqc, ps_var)
                nc.sync.dma_start(out=cc_in[0:1, C0 : C0 + CHUNK], in_=ssqc)

            # ---- variance all-reduce + rsqrt --------------------------
            nc.gpsimd.collective_compute(
                "AllReduce",
                mybir.AluOpType.add,
                replica_groups=[list(range(NCORES))],
                ins=[cc_in[:].opt()],
                outs=[cc_out[:].opt()],
            )
            nc.sync.dma_start(
                out=rs64, in_=cc_out[0:1, :].rearrange("a (p j) -> (a p) j", p=128)
            )
            nc.scalar.activation(tmp64, rs64, AF.Sqrt, bias=eps_sb[:, 0:1], scale=1.0 / HID)
            nc.vector.reciprocal(r64, tmp64)
            # r64[q, j] holds r for s = q*64+j; bounce through DRAM to
            # replicate r across all 128 partitions.
            r_dram = dram.tile([1, N], F32)
            nc.sync.dma_start(
                out=r_dram[0:1, :].rearrange("a (p j) -> (a p) j", p=128), in_=r64
            )
            # ---- out projection (row-parallel partial, transposed) ----
            for c in range(NCHUNK):
                C0 = c * CHUNK
                rbc = osbp.tile([128, CHUNK], F32, tag="rbc", name="rbc", bufs=2)
                nc.sync.dma_start(
                    out=rbc, in_=r_dram[0:1, C0 : C0 + CHUNK].to_broadcast([128, CHUNK])
                )
                for t in range(2):
                    nc.vector.tensor_mul(
                        gh_sb[:, t, C0 : C0 + CHUNK], gh_sb[:, t, C0 : C0 + CHUNK], rbc
                    )
                for mt in range(HID // 128):
                    pj = psA.tile([128, CHUNK], F32, tag="pj", name="pjo")
                    for kt in range(2):
                        nc.tensor.matmul(
                            pj,
                            lhsT=wo_sb[:, kt, mt * 128 : (mt + 1) * 128],
                            rhs=gh_sb[:, kt, C0 : C0 + CHUNK],
                            start=(kt == 0),
                            stop=(kt == 1),
                        )
                    osb = osbp.tile([128, CHUNK], F32, tag="osb", name="osb")
                    nc.scalar.copy(osb, pj)
                    nc.sync.dma_start(
                        out=outp_d[mt * 128 : (mt + 1) * 128, C0 : C0 + CHUNK], in_=osb
                    )

    nc.finalize()
    return nc


def _prep_inputs(hidden_states, kv_cache, W_qkv, W_gate, W_out, norm_weight):
    hsT = np.ascontiguousarray(hidden_states.T).astype(BF)
    in_maps = []
    arr = np.arange(BLOCK, dtype=np.float32) + 1.0  # 1..256
    nloc = np.arange(128, dtype=np.float32)
    for c in range(NCORES):
        heads = [4 * c + h for h in range(HL)]
        # fused weight: [Q(4x64), K(4x64), V(4x64), gate(256)] x HID
        rows = []
        for part in range(3):  # q, k, v
            for g in heads:
                base = g * 3 * D + part * D
                rows.append(W_qkv[base : base + D])
        rows.append(W_gate[c * 256 : (c + 1) * 256])
        w_comb = np.concatenate(rows, axis=0)  # [1024, HID]
        wcomb = np.ascontiguousarray(w_comb.T).astype(BF)  # [HID, 1024]

        w_out_c = W_out[:, c * 256 : (c + 1) * 256] * norm_weight[c * 256 : (c + 1) * 256][None, :]
        wout = np.ascontiguousarray(w_out_c.T).astype(BF)  # [256, HID]

        s = SLOPE[heads]  # [4]
        qd = np.zeros((128, HL // 2, BLOCK), np.float32)
        kd = np.zeros((128, 2 * HL), np.float32)
        dd = np.zeros((128, 2 * HL, BLOCK), np.float32)
        bd = np.zeros((128, HL // 2), np.float32)
        kv0 = np.zeros((128, HL // 2, D), np.float32)
        for h in range(HL):
            sh = s[h]
            b = (h % 2) * 64
            p = h // 2
            qd[b : b + 64, p, :] = np.exp(-sh * arr)[None, :]
            bd[b : b + 64, p] = math.exp(-sh * BLOCK)
            kv0[b : b + 64, p, :] = kv_cache[heads[h]]
            for half in range(2):
                npos = half * 128 + nloc
                kd[:, 2 * h + half] = np.exp(-sh * (BLOCK - (npos + 1)))
                idx = arr[None, :] - 1 - npos[:, None]  # m - n
                dd[:, 2 * h + half, :] = np.where(idx >= 0, np.exp(-sh * idx), 0.0)
        in_maps.append(
            {
                "hsT": hsT,
                "wcomb": wcomb,
                "wout": wout,
                "dd": dd,
                "qd": qd.astype(BF),
                "kd": kd,
                "bd": bd,
                "kv0": kv0,
            }
        )
    return in_maps


def kernel(**inputs):
    global _NC_CACHE, LAST_EXEC_NS, LAST_RESULTS
    hidden_states = np.asarray(inputs["hidden_states"], dtype=np.float32)
    kv_cache = np.asarray(inputs["kv_cache"], dtype=np.float32)
    W_qkv = np.asarray(inputs["W_qkv"], dtype=np.float32)
    W_gate = np.asarray(inputs["W_gate"], dtype=np.float32)
    W_out = np.asarray(inputs["W_out"], dtype=np.float32)
    norm_weight = np.asarray(inputs["norm_weight"], dtype=np.float32)

    if _NC_CACHE is None:
        _NC_CACHE = _build_module()
    nc = _NC_CACHE

    in_maps = _prep_inputs(hidden_states, kv_cache, W_qkv, W_gate, W_out, norm_weight)
    res = run_bass_kernel_spmd(nc, in_maps, core_ids=list(range(NCORES)))
    LAST_EXEC_NS = res.exec_time_ns
    LAST_RESULTS = res
    acc = res.results[0]["outp"].astype(np.float64)
    for c in range(1, NCORES):
        acc += res.results[c]["outp"]
    return np.ascontiguousarray(acc.T).astype(np.float32)
